# revision 1
# baseline (speedup 1.0000x reference)
"""
LoRA-Quant-Linear Trainium2 kernel (8 NeuronCores).

Math:  out = x @ W^T + bias + LORA_SCALE * ((x @ a^T) @ b^T)
       a = qa * scale_a  [16, 4096],  b = qb * scale_b  [4096, 16]

Sharding (2 batch-groups x 4 out-column-groups = 8 cores):
  core c = (mg, ng), mg = c // 4, ng = c % 4
    - x rows   [mg*8192 : (mg+1)*8192]  (of B*S = 16384), host-transposed -> xT [4096, 8192]
    - W rows   [ng*1024 : (ng+1)*1024]  (out_features),   host-transposed -> wT [4096, 1024]
  LoRA is folded into the weight chunk on the host (out = x @ (W^T + s*a^T b^T) + bias,
  exact associativity; the fold is 0.4% of the FLOPs).  The W chunk stays resident in
  SBUF (128 KiB/partition) and the kernel streams x slivers through it.
  float32r (fp22 multiply, fp32 accumulate) runs the PE at full rate for N>=256;
  tensors feeding the PE are declared float32r end-to-end (walrus requires f32r
  operands to be produced as f32r; numpy side is still float32).
"""

import numpy as np

LORA_SCALE = 32.0 / 16.0

P = 128
K = 4096            # contraction dim (D_in)
KT = K // P         # 32 k-tiles
M_CORE = 8192       # x rows per core
N_CORE = 1024       # out columns per core
MT = M_CORE // P    # 64 m-slivers
NB = 512            # moving free dim per matmul (PSUM bank = 512 f32)
NH = N_CORE // NB   # 2
N_CORES = 8
MG, NG = 2, 4       # core grid

_CACHE = {}


def _build_program():
    import concourse.tile as tile
    from concourse import bacc, mybir
    from contextlib import ExitStack

    f32 = mybir.dt.float32
    f32r = mybir.dt.float32r

    nc = bacc.Bacc("TRN2", target_bir_lowering=False, debug=False,
                   num_devices=N_CORES)

    # host-pretiled layouts: xT[mt, p, kt, ml] = x[mt*128+ml, kt*128+p]
    # (per-sliver contiguous => 16 KiB/partition DMA lines), and
    # wT[p, kt, n] = W_eff^T[kt*128+p, n] (one full-rate DMA).
    xT = nc.dram_tensor("xT", [MT, P, KT, P], f32r, kind="ExternalInput").ap()
    wT = nc.dram_tensor("wT", [P, KT, N_CORE], f32r, kind="ExternalInput").ap()
    biasb = nc.dram_tensor("biasb", [P, N_CORE], f32, kind="ExternalInput").ap()
    out = nc.dram_tensor("out", [M_CORE, N_CORE], f32, kind="ExternalOutput").ap()

    out_t = out.rearrange("(mt p) n -> mt p n", p=P)    # [64, 128, 1024]

    with tile.TileContext(nc) as tc, ExitStack() as ctx:
        wpool = ctx.enter_context(tc.tile_pool(name="wres", bufs=1))
        cpool = ctx.enter_context(tc.tile_pool(name="consts", bufs=1))
        xpool = ctx.enter_context(tc.tile_pool(name="xs", bufs=3))
        opool = ctx.enter_context(tc.tile_pool(name="outs", bufs=2))
        pspool = ctx.enter_context(tc.tile_pool(name="ps", bufs=8, space="PSUM"))

        # resident fused weights [128, kt, n]
        w_sb = wpool.tile([P, KT, N_CORE], f32r)
        for kt in range(KT):
            nc.sync.dma_start(w_sb[:, kt, :], wT[:, kt, :])

        bias_sb = cpool.tile([P, N_CORE], f32)
        nc.scalar.dma_start(bias_sb[:], biasb)

        # main GEMM: stream x slivers, accumulate 32 k-tiles into 2 PSUM banks
        for mt in range(MT):
            x_sb = xpool.tile([P, KT, P], f32r, tag="x", name=f"x_{mt}")
            nc.scalar.dma_start(x_sb[:], xT[mt])
            pss = [pspool.tile([P, NB], f32, tag="ps", name=f"ps_{mt}_{i}")
                   for i in range(NH)]
            for kt in range(KT):
                for nh in range(NH):
                    nc.tensor.matmul(
                        pss[nh][:],
                        x_sb[:, kt, :],
                        w_sb[:, kt, nh * NB:(nh + 1) * NB],
                        start=(kt == 0), stop=(kt == KT - 1),
                    )
            o_sb = opool.tile([P, N_CORE], f32, tag="o", name=f"o_{mt}")
            for nh in range(NH):
                nc.vector.tensor_add(
                    o_sb[:, nh * NB:(nh + 1) * NB],
                    pss[nh][:],
                    bias_sb[:, nh * NB:(nh + 1) * NB],
                )
            nc.sync.dma_start(out_t[mt], o_sb[:])

    nc.compile()
    return nc


def _get_program():
    if "nc" not in _CACHE:
        _CACHE["nc"] = _build_program()
    return _CACHE["nc"]


def _make_in_maps(x, W, bias, qa, qb, scale_a, scale_b):
    x2 = np.ascontiguousarray(x.reshape(MG * M_CORE, K))
    a_deq = qa.astype(np.float32) * np.float32(scale_a)       # [16, 4096]
    b_deq = qb.astype(np.float32) * np.float32(scale_b)       # [4096, 16]
    # W_eff^T = W^T + s * a^T @ b^T   -> [K, N_full]
    w_eff_T = W.T + np.float32(LORA_SCALE) * (a_deq.T @ b_deq.T)
    bias = bias.astype(np.float32)

    # [mt, ml, kt, p] -> [mt, p, kt, ml]
    xT_by_mg = [np.ascontiguousarray(
                    x2[mg * M_CORE:(mg + 1) * M_CORE, :]
                    .reshape(MT, P, KT, P).transpose(0, 3, 2, 1))
                for mg in range(MG)]
    in_maps = []
    for c in range(N_CORES):
        mg, ng = c // NG, c % NG
        nsl = slice(ng * N_CORE, (ng + 1) * N_CORE)
        in_maps.append({
            "xT": xT_by_mg[mg],
            "wT": np.ascontiguousarray(
                w_eff_T[:, nsl].reshape(KT, P, N_CORE).transpose(1, 0, 2)),
            "biasb": np.ascontiguousarray(
                np.broadcast_to(bias[nsl], (P, N_CORE))),
        })
    return in_maps


def kernel(x, W, bias, qa, qb, scale_a, scale_b, _trace=False):
    from concourse.bass_utils import run_bass_kernel_spmd

    nc = _get_program()
    in_maps = _make_in_maps(np.asarray(x, dtype=np.float32),
                            np.asarray(W, dtype=np.float32),
                            np.asarray(bias, dtype=np.float32),
                            np.asarray(qa), np.asarray(qb),
                            np.asarray(scale_a), np.asarray(scale_b))
    res = run_bass_kernel_spmd(nc, in_maps, core_ids=list(range(N_CORES)),
                               trace=_trace)
    B, S = 4, 4096
    full = np.empty((MG * M_CORE, NG * N_CORE), dtype=np.float32)
    for c in range(N_CORES):
        mg, ng = c // NG, c % NG
        full[mg * M_CORE:(mg + 1) * M_CORE,
             ng * N_CORE:(ng + 1) * N_CORE] = res.results[c]["out"]
    if _trace:
        kernel._last_results = res
    return full.reshape(B, S, K)

